# revision 17
# baseline (speedup 1.0000x reference)
"""BiMamba (bidirectional Mamba-1 selective scan) on 8 Trainium2 NeuronCores.

Sharding: core c = (b, dir, half) with b = c>>2, dir = (c>>1)&1, half = c&1.

Per-exec host<->device staging is the dominant cost on this platform, so
inputs are bf16 and deduplicated across cores: each core stages only
  - half of its (b, dir) x (transposed, pre-flipped for bwd)   [2 MB]
  - half of its (dir, half) big-weight blob                    [3.2 MB]
  - a small f32 per-channel parameter pack                     [94 KB]
and two on-device AllGathers reassemble the full tensors:
  - x      over pairs [[0,1],[2,3],[4,5],[6,7]]  (same (b, dir))
  - weights over pairs [[0,4],[1,5],[2,6],[3,7]] (same (dir, half))

Compute per core in a transposed [d, L] layout:
  in_proj (bf16 matmuls) -> depthwise conv (shifted per-partition scalar
  muls) -> silu -> x_proj partial -> pairwise AllReduce of x_dbl [96, L]
  -> dt softplus -> selective scan (exp on ACT, mults on GPSIMD,
  tensor_tensor_scan on DVE) -> gate with silu(z) -> out_proj partial
  (bf16) -> pairwise ReduceScatter -> each core outputs its half of the
  summed [DM, L] as bf16 [DM/2, L].
Host assembles/transposes/flips the 8 bf16 partials into the full output.
"""
import sys
sys.path.insert(0, "/opt/trn_rl_repo")
import numpy as np
from contextlib import ExitStack

import concourse.bass as bass
import concourse.mybir as mybir
import concourse.tile as tile
from concourse.vector_clock import ScopedClock

F32 = mybir.dt.float32
F32R = mybir.dt.float32r
BF16 = mybir.dt.bfloat16
AF = mybir.ActivationFunctionType
OP = mybir.AluOpType

# ---------------------------------------------------------------- geometry
B, L, DM = 2, 2048, 1024
DI, DS, DC, DTR = 2 * DM, 16, 4, DM // 16
DH = DI // 2              # d_inner half per core
NT = DH // 128            # d-tiles per core
HALVES = 2
LC = L // HALVES          # L chunk per phase
MMT = 512                 # matmul free-dim tile

# flat bf16 weight blob layout (per (dir, half) combo)
SZ_WIN = DM * 2 * DH      # w_in^T  [DM, 2*DH]
SZ_WOUT = DH * DM         # w_out^T [DH, DM]
SZ_WX = DH * 96           # w_x^T   [DH, 96]
SZ_WDT = DTR * DH         # w_dt^T  [DTR, DH]
OFF_WIN = 0
OFF_WOUT = OFF_WIN + SZ_WIN
OFF_WX = OFF_WOUT + SZ_WOUT
OFF_WDT = OFF_WX + SZ_WX
OFF_SM = OFF_WDT + SZ_WDT
XTOT = DM * L             # 2,097,152
XCHUNK = XTOT // 2

# smalls [128, SC] f32 column layout
SC_CONVW = 0              # NT*DC cols, col = nt*DC + k
SC_CONVB = SC_CONVW + NT * DC
SC_DTB = SC_CONVB + NT
SC_A = SC_DTB + NT        # NT*DS cols, col = nt*DS + n
SC_D = SC_A + NT * DS
SC = SC_D + NT            # 184
WTOT = OFF_SM + 2 * 128 * SC  # smalls f32 stored as u16 pairs
WCHUNK = WTOT // 2

MAXW = 1                  # codegen limit: sem waits per instruction


# ------------------------------------------------------------- tile patch
def _patched_drain_and_barrier(self, tick_clock, wait_clock):
    nop_inst = self.nc.sync.nop(nofuse=True)
    wait_clock.add_sem_waits(
        nop_inst.ins, ScopedClock({None: tick_clock.global_clock}))
    si = nop_inst.ins.sync_info
    if si is not None and si.on_wait and len(si.on_wait) > MAXW:
        extra = list(si.on_wait[MAXW:])
        del si.on_wait[MAXW:]
        for i in range(0, len(extra), MAXW):
            nop2 = self.nc.sync.nop(nofuse=True)
            nop2.ins.sync_info = mybir.SyncInfo(
                on_wait=extra[i:i + MAXW], on_update=[])
    self.nc.sync.drain()
    self.nc.all_engine_barrier()
    assert self.sems is not None
    popped = self.nc._tile_sem_poison_stack.pop()
    assert popped is self._sem_poison
    self.nc.clear_and_free_semaphores(list(self.sems.allocated().values()))
    self.nc.all_engine_barrier()


tile.TileContext._drain_and_barrier = _patched_drain_and_barrier


def split_multiwaits(nc, maxw=MAXW):
    ctr = 0
    for fn in nc.m.functions:
        for blk in fn.blocks:
            il = list(blk.instructions)
            out = []
            changed = False
            for ins in il:
                si = getattr(ins, "sync_info", None)
                waits = list(si.on_wait) if (si is not None and si.on_wait) else []
                if len(waits) > maxw:
                    changed = True
                    extra, keep = waits[:-maxw], waits[-maxw:]
                    for i in range(0, len(extra), maxw):
                        nop = mybir.InstNoOp(name=f"wsplit_{ctr}", ins=[], outs=[])
                        ctr += 1
                        nop.engine = ins.engine
                        nop.sync_info = mybir.SyncInfo(
                            on_wait=extra[i:i + maxw], on_update=[])
                        out.append(nop)
                    si.on_wait = keep
                out.append(ins)
            if changed:
                blk.instructions = out
    return ctr


# ------------------------------------------------------------ bass builder
def build_nc():
    nc = bass.Bass()
    P = 128
    LTN = LC // MMT       # matmul L-tiles per half
    KT = DM // P          # d_model tiles (in_proj contraction, out rows)

    xch_d = nc.declare_dram_parameter("xch", [1, XCHUNK], BF16, isOutput=False)
    wch_d = nc.declare_dram_parameter("wch", [1, WCHUNK], BF16, isOutput=False)
    outp_d = nc.declare_dram_parameter("outp", [DM // 2, L], BF16, isOutput=True)

    # gather scratch
    xg_in = nc.dram_tensor("xg_in", [1, XCHUNK], BF16)
    xg_out = nc.dram_tensor("xg_out", [1, XTOT], BF16)
    wg_in = nc.dram_tensor("wg_in", [1, WCHUNK], BF16)
    wg_out = nc.dram_tensor("wg_out", [1, WTOT], BF16)
    groups_x = [[0, 1], [2, 3], [4, 5], [6, 7]]
    groups_w = [[0, 4], [1, 5], [2, 6], [3, 7]]

    ccin = [nc.dram_tensor(f"ccin{h}", [96, LC], F32) for h in range(HALVES)]
    ccout = [nc.dram_tensor(f"ccout{h}", [96, LC], F32) for h in range(HALVES)]

    rs_in = nc.dram_tensor("rs_in", [DM, L], BF16)
    rs_out = nc.dram_tensor("rs_out", [DM // 2, L], BF16)

    with tile.TileContext(nc) as tc, ExitStack() as ctx:
        pool = ctx.enter_context(tc.tile_pool(name="sb", bufs=1))
        psum = ctx.enter_context(tc.tile_pool(name="ps", bufs=6, space="PSUM"))

        # ---- gathers: extern chunk -> internal -> AllGather -> full
        dx0 = nc.sync.dma_start(xg_in[:], xch_d[:])
        ccx = nc.gpsimd.collective_compute(
            "AllGather", OP.bypass, replica_groups=groups_x,
            ins=[xg_in[:]], outs=[xg_out[:]])
        tile.add_dep_helper(ccx.ins, dx0.ins, reason="x gather after stage-in")
        dw0 = nc.sync.dma_start(wg_in[:], wch_d[:])
        ccw = nc.gpsimd.collective_compute(
            "AllGather", OP.bypass, replica_groups=groups_w,
            ins=[wg_in[:]], outs=[wg_out[:]])
        tile.add_dep_helper(ccw.ins, dw0.ins, reason="w gather after stage-in")

        # shaped views of the gathered blobs
        xv = xg_out[0, :].rearrange("(k l) -> k l", l=L)            # [DM, L]
        wv_in = wg_out[0, OFF_WIN:OFF_WIN + SZ_WIN].rearrange(
            "(k m) -> k m", m=2 * DH)                               # [DM, 2DH]
        wv_out = wg_out[0, OFF_WOUT:OFF_WOUT + SZ_WOUT].rearrange(
            "(k m) -> k m", m=DM)                                   # [DH, DM]
        wv_x = wg_out[0, OFF_WX:OFF_WX + SZ_WX].rearrange(
            "(k m) -> k m", m=96)                                   # [DH, 96]
        wv_dt = wg_out[0, OFF_WDT:OFF_WDT + SZ_WDT].rearrange(
            "(k m) -> k m", m=DH)                                   # [DTR, DH]

        # resident small params (f32 stored as u16 pairs in the w blob)
        smalls = pool.tile([P, SC], F32, tag="smalls")
        dsm = nc.sync.dma_start(
            smalls[:],
            wg_out[0, OFF_SM:OFF_SM + 2 * P * SC].bitcast(F32).rearrange(
                "(p c) -> p c", c=SC))
        tile.add_dep_helper(dsm.ins, ccw.ins, reason="read after w gather")
        convw_sb = smalls[:, SC_CONVW:SC_CONVW + NT * DC]
        cb_sb = smalls[:, SC_CONVB:SC_CONVB + NT]
        dtb_sb = smalls[:, SC_DTB:SC_DTB + NT]
        a_sb = smalls[:, SC_A:SC_A + NT * DS]
        dcol_sb = smalls[:, SC_D:SC_D + NT]

        # resident small weights (bf16 load -> f32r convert)
        wx_bf = pool.tile([P, NT, 96], BF16, tag="wxb")
        d = nc.sync.dma_start(wx_bf[:], wv_x.rearrange("(kt p) m -> p kt m", p=P))
        tile.add_dep_helper(d.ins, ccw.ins, reason="read after w gather")
        wx_r = pool.tile([P, NT, 96], F32R, tag="wx")
        nc.scalar.copy(wx_r[:], wx_bf[:])
        wdt_bf = pool.tile([DTR, NT, P], BF16, tag="wdtb")
        d = nc.sync.dma_start(wdt_bf[:],
                              wv_dt.rearrange("k (mt m) -> k mt m", m=P))
        tile.add_dep_helper(d.ins, ccw.ins, reason="read after w gather")
        wdt_r = pool.tile([DTR, NT, P], F32R, tag="wdt")
        nc.scalar.copy(wdt_r[:], wdt_bf[:])

        halo = [pool.tile([P, DC - 1], F32, tag=f"halo{nt}", name=f"halo{nt}")
                for nt in range(NT)]
        states = pool.tile([P, DS * NT], F32, tag="states")

        xt_re = xv.rearrange("(kt p) l -> p kt l", p=P)

        for half in range(HALVES):
            l0 = half * LC
            # -------- stage 1+2: in_proj -> xi -> conv (shifts) -> u; z -> sz
            xt_t = []
            for kt in range(KT):
                t = pool.tile([P, LC], BF16, tag="xtl", bufs=8)
                d = nc.sync.dma_start(t[:], xt_re[:, kt, l0:l0 + LC])
                tile.add_dep_helper(d.ins, ccx.ins, reason="read after x gather")
                xt_t.append(t)
            u_t = []
            sz_t = []
            for mt in range(2 * NT):
                win_t = pool.tile([P, KT, P], BF16, tag="wst", bufs=2)
                d = nc.sync.dma_start(
                    win_t[:],
                    wv_in[:, mt * P:(mt + 1) * P].rearrange(
                        "(kt p) q -> p kt q", p=P))
                tile.add_dep_helper(d.ins, ccw.ins, reason="read after w gather")
                if mt < NT:
                    xi = pool.tile([P, DC - 1 + LC], F32, tag="xi", bufs=2)
                else:
                    sz = pool.tile([P, LC], BF16, tag="sz", bufs=8)
                    sz_t.append(sz)
                for lt in range(LTN):
                    acc = psum.tile([P, MMT], F32, tag="mm")
                    for kt in range(KT):
                        nc.tensor.matmul(
                            acc[:], win_t[:, kt, :],
                            xt_t[kt][:, lt * MMT:(lt + 1) * MMT],
                            start=(kt == 0), stop=(kt == KT - 1))
                    if mt < NT:
                        nc.scalar.copy(
                            xi[:, DC - 1 + lt * MMT:DC - 1 + (lt + 1) * MMT],
                            acc[:])
                    else:
                        nc.scalar.activation(
                            sz_t[mt - NT][:, lt * MMT:(lt + 1) * MMT],
                            acc[:], AF.Silu)
                if mt >= NT:
                    continue
                # depthwise causal conv via shifted per-partition scalar muls
                nt = mt
                if half == 0:
                    nc.vector.memset(halo[nt][:], 0.0)
                nc.vector.tensor_copy(xi[:, 0:DC - 1], halo[nt][:])
                u = pool.tile([P, LC], F32R, tag="u", bufs=8)
                for lt in range(LTN):
                    t0 = pool.tile([P, MMT], F32, tag="cv", bufs=3)
                    nc.scalar.mul(t0[:],
                                  xi[:, lt * MMT:lt * MMT + MMT],
                                  convw_sb[:, nt * DC:nt * DC + 1])
                    t1 = pool.tile([P, MMT], F32, tag="cv", bufs=3)
                    nc.scalar.mul(t1[:],
                                  xi[:, lt * MMT + 1:lt * MMT + 1 + MMT],
                                  convw_sb[:, nt * DC + 1:nt * DC + 2])
                    a01 = pool.tile([P, MMT], F32, tag="cv2", bufs=3)
                    nc.vector.tensor_tensor(a01[:], t0[:], t1[:], OP.add)
                    t2 = pool.tile([P, MMT], F32, tag="cv", bufs=3)
                    nc.scalar.mul(t2[:],
                                  xi[:, lt * MMT + 2:lt * MMT + 2 + MMT],
                                  convw_sb[:, nt * DC + 2:nt * DC + 3])
                    t3 = pool.tile([P, MMT], F32, tag="cv", bufs=3)
                    nc.scalar.mul(t3[:],
                                  xi[:, lt * MMT + 3:lt * MMT + 3 + MMT],
                                  convw_sb[:, nt * DC + 3:nt * DC + 4])
                    a23 = pool.tile([P, MMT], F32, tag="cv2", bufs=3)
                    nc.vector.tensor_tensor(a23[:], t2[:], t3[:], OP.add)
                    cacc = pool.tile([P, MMT], F32, tag="cv2", bufs=3)
                    nc.vector.tensor_tensor(cacc[:], a01[:], a23[:], OP.add)
                    nc.scalar.activation(
                        u[:, lt * MMT:(lt + 1) * MMT], cacc[:], AF.Silu,
                        bias=cb_sb[:, nt:nt + 1])
                # save halo for next half (before xi slot recycles)
                nc.vector.tensor_copy(halo[nt][:], xi[:, LC:LC + DC - 1])
                u_t.append(u)

            # ---------------- stage 3: x_proj partial [96, LC]
            xdblp = pool.tile([96, LC], F32, tag="xdblp")
            for lt in range(LTN):
                acc96 = psum.tile([96, MMT], F32, tag="mm96", bufs=2)
                for nt in range(NT):
                    nc.tensor.matmul(
                        acc96[:], wx_r[:, nt, :],
                        u_t[nt][:, lt * MMT:(lt + 1) * MMT],
                        start=(nt == 0), stop=(nt == NT - 1))
                nc.scalar.copy(xdblp[:, lt * MMT:(lt + 1) * MMT], acc96[:])

            # ---------------- stage 4: pairwise AllReduce of x_dbl
            dma_in = nc.sync.dma_start(ccin[half][:], xdblp[:])
            cc = nc.gpsimd.collective_compute(
                "AllReduce", OP.add, replica_groups=groups_x,
                ins=[ccin[half][:]], outs=[ccout[half][:]])
            tile.add_dep_helper(cc.ins, dma_in.ins, reason="cc after dma_in")

            # ---------------- stage 5: dt = softplus(Wdt@dtr + b); dtu; y=D*u
            xdbl = pool.tile([96, LC], F32, tag="xdbl")
            dma_out = nc.sync.dma_start(xdbl[:], ccout[half][:])
            tile.add_dep_helper(dma_out.ins, cc.ins, reason="read after cc")
            dtr_r = pool.tile([DTR, LC], F32R, tag="dtr")
            nc.scalar.copy(dtr_r[:], xdbl[0:DTR, :])
            dt_t = []
            for nt in range(NT):
                dt = pool.tile([P, LC], F32, tag="dt", bufs=8)
                for lt in range(LTN):
                    acc = psum.tile([P, MMT], F32, tag="mm")
                    nc.tensor.matmul(
                        acc[:], wdt_r[:, nt, :],
                        dtr_r[:, lt * MMT:(lt + 1) * MMT],
                        start=True, stop=True)
                    e = pool.tile([P, MMT], F32, tag="spe", bufs=2)
                    nc.scalar.activation(e[:], acc[:], AF.Exp,
                                         bias=dtb_sb[:, nt:nt + 1])
                    nc.scalar.activation(
                        dt[:, lt * MMT:(lt + 1) * MMT], e[:], AF.Ln, bias=1.0)
                dt_t.append(dt)

            y_t = []
            for nt in range(NT):
                y = pool.tile([P, LC], F32, tag="y", bufs=8)
                ufp = u_t[nt][:].bitcast(F32)
                nc.scalar.mul(y[:], ufp, dcol_sb[:, nt:nt + 1])      # y=D*u
                nc.vector.tensor_tensor(u_t[nt][:], dt_t[nt][:], ufp,
                                        OP.mult)  # dtu in place
                y_t.append(y)

            # ---------------- stage 6: selective scan
            for n in range(DS):
                Bb = pool.tile([P, LC], F32, tag="bc", bufs=3)
                nc.sync.dma_start(
                    Bb[:], ccout[half][DTR + n:DTR + n + 1, :]
                    .partition_broadcast(P))
                Cb = pool.tile([P, LC], F32, tag="bc", bufs=3)
                nc.sync.dma_start(
                    Cb[:], ccout[half][DTR + DS + n:DTR + DS + n + 1, :]
                    .partition_broadcast(P))
                for nt in range(NT):
                    dA = pool.tile([P, LC], F32, tag="tr", bufs=3)
                    nc.scalar.activation(
                        dA[:], dt_t[nt][:], AF.Exp,
                        scale=a_sb[:, nt * DS + n:nt * DS + n + 1])
                    dBu = pool.tile([P, LC], F32, tag="tr", bufs=3)
                    nc.gpsimd.tensor_tensor(
                        dBu[:], u_t[nt][:].bitcast(F32), Bb[:], OP.mult)
                    h = pool.tile([P, LC], F32, tag="tr", bufs=3)
                    init = 0.0 if half == 0 else states[:, n * NT + nt:
                                                        n * NT + nt + 1]
                    nc.vector.tensor_tensor_scan(
                        h[:], dA[:], dBu[:], init, OP.mult, OP.add)
                    if half < HALVES - 1:
                        nc.scalar.copy(
                            states[:, n * NT + nt:n * NT + nt + 1],
                            h[:, LC - 1:LC])
                    nc.gpsimd.tensor_tensor(h[:], h[:], Cb[:], OP.mult)
                    nc.vector.tensor_tensor(y_t[nt][:], y_t[nt][:], h[:],
                                            OP.add)

            # ---------------- stage 7: gate + out_proj partial (bf16)
            yg_t = []
            for nt in range(NT):
                yg = pool.tile([P, LC], BF16, tag="xtl", bufs=8)
                nc.vector.tensor_tensor(yg[:], y_t[nt][:], sz_t[nt][:], OP.mult)
                yg_t.append(yg)
            rs_writes = []
            for mt in range(KT):
                wout_t = pool.tile([P, NT, P], BF16, tag="wst", bufs=2)
                d = nc.sync.dma_start(
                    wout_t[:],
                    wv_out[:, mt * P:(mt + 1) * P].rearrange(
                        "(kt p) q -> p kt q", p=P))
                tile.add_dep_helper(d.ins, ccw.ins, reason="read after w gather")
                for lt in range(LTN):
                    acc = psum.tile([P, MMT], F32, tag="mm")
                    for kt in range(NT):
                        nc.tensor.matmul(
                            acc[:], wout_t[:, kt, :],
                            yg_t[kt][:, lt * MMT:(lt + 1) * MMT],
                            start=(kt == 0), stop=(kt == NT - 1))
                    o = pool.tile([P, MMT], BF16, tag="op", bufs=2)
                    nc.scalar.copy(o[:], acc[:])
                    w = nc.sync.dma_start(
                        rs_in[mt * P:(mt + 1) * P,
                              l0 + lt * MMT:l0 + (lt + 1) * MMT], o[:])
                    rs_writes.append(w)

            if half == 0:
                rs_writes_h0 = rs_writes

        # ---------------- stage 8: pairwise ReduceScatter of out partials
        rs = nc.gpsimd.collective_compute(
            "ReduceScatter", OP.add, replica_groups=groups_x,
            ins=[rs_in[:]], outs=[rs_out[:]])
        for w in rs_writes_h0 + rs_writes:
            tile.add_dep_helper(rs.ins, w.ins, reason="rs after partial write")
        dout = nc.sync.dma_start(outp_d[:], rs_out[:])
        tile.add_dep_helper(dout.ins, rs.ins, reason="out after rs")

    split_multiwaits(nc)
    return nc


# ------------------------------------------------------------- host side
def _prep_core_inputs(inputs, b, dir_, half):
    import ml_dtypes
    bf16 = ml_dtypes.bfloat16
    pre = "f_" if dir_ == 0 else "b_"
    sl = slice(half * DH, (half + 1) * DH)

    # x chunk: transposed (pre-flipped for bwd), shared by the (b, dir) pair
    x = np.asarray(inputs["x"][b], dtype=np.float32)          # [L, DM]
    if dir_ == 1:
        x = x[::-1]
    xt = np.ascontiguousarray(x.T).astype(bf16).reshape(-1)   # [DM*L]
    xch = xt[half * XCHUNK:(half + 1) * XCHUNK]

    # weight blob: per (dir, half) combo, split across the two batches
    w_in_full = np.asarray(inputs[pre + "in_proj_w"], np.float32)  # [2DI, DM]
    w_in = np.concatenate([w_in_full[sl], w_in_full[DI + half * DH:
                                                    DI + (half + 1) * DH]], 0)
    w_out = np.asarray(inputs[pre + "out_proj_w"], np.float32)[:, sl]  # [DM,DH]
    w_x = np.asarray(inputs[pre + "x_proj_w"], np.float32)[:, sl]   # [96, DH]
    w_dt = np.asarray(inputs[pre + "dt_proj_w"], np.float32)[sl]    # [DH, DTR]
    conv_w = np.asarray(inputs[pre + "conv_w"], np.float32)[sl, 0]  # [DH, DC]
    conv_b = np.asarray(inputs[pre + "conv_b"], np.float32)[sl]
    dt_b = np.asarray(inputs[pre + "dt_proj_b"], np.float32)[sl]
    A = -np.exp(np.asarray(inputs[pre + "A_log"], np.float32))[sl]  # [DH, DS]
    Dp = np.asarray(inputs[pre + "D"], np.float32)[sl]

    smalls = np.zeros((128, SC), np.float32)
    smalls[:, SC_CONVW:SC_CONVW + NT * DC] = (
        conv_w.reshape(NT, 128, DC).transpose(1, 0, 2).reshape(128, NT * DC))
    smalls[:, SC_CONVB:SC_CONVB + NT] = conv_b.reshape(NT, 128).T
    smalls[:, SC_DTB:SC_DTB + NT] = dt_b.reshape(NT, 128).T
    smalls[:, SC_A:SC_A + NT * DS] = (
        A.reshape(NT, 128, DS).transpose(1, 0, 2).reshape(128, NT * DS))
    smalls[:, SC_D:SC_D + NT] = Dp.reshape(NT, 128).T

    blob = np.concatenate([
        np.ascontiguousarray(w_in.T).astype(bf16).reshape(-1),
        np.ascontiguousarray(w_out.T).astype(bf16).reshape(-1),
        np.ascontiguousarray(w_x.T).astype(bf16).reshape(-1),
        np.ascontiguousarray(w_dt.T).astype(bf16).reshape(-1),
        np.ascontiguousarray(smalls).view(np.uint16).view(bf16).reshape(-1),
    ])
    wch = blob[b * WCHUNK:(b + 1) * WCHUNK]

    return {
        "xch": np.ascontiguousarray(xch).reshape(1, XCHUNK),
        "wch": np.ascontiguousarray(wch).reshape(1, WCHUNK),
    }


_CACHE = {}


def _get_nc():
    if "nc" not in _CACHE:
        _CACHE["nc"] = build_nc()
    return _CACHE["nc"]


def _make_runner():
    """Jitted 8-core PJRT runner (no donation so it can be re-invoked for
    timing). Returns (fn, in_names, out_names, out_avals)."""
    import jax
    from jax.sharding import Mesh, PartitionSpec
    from jax.experimental.shard_map import shard_map
    from concourse import bass2jax
    from concourse.bass2jax import _bass_exec_p, install_neuronx_cc_hook

    install_neuronx_cc_hook()
    nc = _get_nc()
    pname = nc.partition_id_tensor.name if nc.partition_id_tensor else None
    in_names, out_names, out_avals = [], [], []
    for alloc in nc.m.functions[0].allocations:
        if not isinstance(alloc, mybir.MemoryLocationSet):
            continue
        name = alloc.memorylocations[0].name
        if alloc.kind == "ExternalInput":
            if name != pname:
                in_names.append(name)
        elif alloc.kind == "ExternalOutput":
            out_names.append(name)
            out_avals.append(jax.core.ShapedArray(
                tuple(alloc.tensor_shape), mybir.dt.np(alloc.dtype)))
    all_names = in_names + out_names
    if pname is not None:
        all_names = all_names + [pname]

    def _body(*args):
        operands = list(args)
        if pname is not None:
            operands.append(bass2jax.partition_id_tensor())
        outs = _bass_exec_p.bind(
            *operands, out_avals=tuple(out_avals), in_names=tuple(all_names),
            out_names=tuple(out_names), lowering_input_output_aliases=(),
            sim_require_finite=False, sim_require_nnan=False, nc=nc)
        return tuple(outs)

    devices = jax.devices()[:8]
    mesh = Mesh(np.asarray(devices), ("core",))
    nin = len(in_names) + len(out_names)
    fn = jax.jit(shard_map(
        _body, mesh=mesh, in_specs=(PartitionSpec("core"),) * nin,
        out_specs=(PartitionSpec("core"),) * len(out_names), check_rep=False),
        keep_unused=True)
    return fn, in_names, out_names, out_avals


def _get_runner():
    if "runner" not in _CACHE:
        _CACHE["runner"] = _make_runner()
    return _CACHE["runner"]


def _concat_inputs(in_maps):
    import jax
    fn, in_names, out_names, out_avals = _get_runner()
    concat = [np.concatenate([np.asarray(m[k]) for m in in_maps], axis=0)
              for k in in_names]
    zeros = [np.zeros((8 * a.shape[0], *a.shape[1:]), a.dtype)
             for a in out_avals]
    return [jax.device_put(a) for a in concat + zeros]


def _run(in_maps):
    import jax
    fn, in_names, out_names, out_avals = _get_runner()
    args = _concat_inputs(in_maps)
    outs = [np.asarray(o) for o in fn(*args)]
    return [
        {k: outs[i].reshape(8, *out_avals[i].shape)[c]
         for i, k in enumerate(out_names)}
        for c in range(8)
    ]


def run_timed(in_maps, iters=10):
    import time as _t
    import jax
    fn, *_ = _get_runner()
    args = _concat_inputs(in_maps)
    jax.block_until_ready(fn(*args))
    times = []
    for _ in range(iters):
        t0 = _t.perf_counter()
        jax.block_until_ready(fn(*args))
        times.append(_t.perf_counter() - t0)
    return min(times)


def make_in_maps(inputs):
    return [
        _prep_core_inputs(inputs, c >> 2, (c >> 1) & 1, c & 1)
        for c in range(8)
    ]


def kernel(**inputs):
    res = _run(make_in_maps(inputs))
    out = np.zeros((B, L, 2 * DM), np.float32)
    for b in range(B):
        for dir_ in range(2):
            c0 = (b << 2) | (dir_ << 1)
            part = np.concatenate(
                [res[c0]["outp"].astype(np.float32),
                 res[c0 + 1]["outp"].astype(np.float32)], axis=0)  # [DM, L]
            if dir_ == 1:
                part = part[:, ::-1]
            out[b, :, dir_ * DM:(dir_ + 1) * DM] = part.T
    return out
